# revision 6
# baseline (speedup 1.0000x reference)
"""Bass/Trainium2 kernel for nn_ConcatAttention (additive attention).

Reference computation (per full problem):
    out_d = input_d @ W_d            # [B, Ld, H]
    out_e = input_e @ W_e            # [B, Le, H]
    t     = tanh(out_d[:,None] + out_e[:,:,None] + b)   # [B, Le, Ld, H]
    energy= t @ v                    # [B, Le, Ld, L]
    out   = transpose(energy, (0,3,2,1))                # [B, L, Ld, Le]

Shapes: B=4, Ld=Le=256, D=512, H=256, L=8.

Sharding: 8 cores = (batch b in 0..3) x (d-half in 0..1). Each core handles
d_local in [0,128), all e, producing out[b, :, d_range, :] = [8, 128, 256].

Per-core pipeline (d-major so the PSUM free dim = e = HBM-contiguous):
  - PE: transpose inputs, project out_dT[h,d], out_ebT[h,e] (bias folded)
  - DVE: S[h, e] = out_ebT + out_dT[:, d]  (tensor_scalar, 2x fp32 mode)
  - ACT: tanh over big [128, 4096] tiles (the bottleneck engine: ~55us floor)
  - PE: energy[l, e] = v.T @ tanh_tile per d, col-group packed 4 d-pairs/bank
  - DVE: PSUM bank -> SBUF, DMA out with 512B-contiguous runs
"""

import numpy as np

B, LD, LE = 4, 256, 256
D, H, L = 512, 256, 8
LD_LOC = 128          # d per core
KC = 4                # 512 / 128 k-chunks
HC = 2                # 256 / 128 h-chunks
SBLK = 16             # d per superblock
N_SBLK = LD_LOC // SBLK
N_BIAS = 2            # d's per (sblk, chunk) done via ACT-bias instead of DVE add
MM_BF16 = True        # bf16 tanh/v for the energy contraction (col tiling needs non-4B dtype)

_CACHE = {}


def _build_nc():
    import concourse.bacc as bacc
    import concourse.mybir as mybir
    import concourse.tile as tile
    from concourse import masks

    fp32 = mybir.dt.float32
    mmdt = mybir.dt.bfloat16 if MM_BF16 else mybir.dt.float32

    nc = bacc.Bacc("TRN2", target_bir_lowering=False, debug=False)

    x_d = nc.dram_tensor("input_d", [LD_LOC, D], fp32, kind="ExternalInput").ap()
    x_e = nc.dram_tensor("input_e", [LE, D], fp32, kind="ExternalInput").ap()
    w_d = nc.dram_tensor("w_d", [D, H], fp32, kind="ExternalInput").ap()
    w_e = nc.dram_tensor("w_e", [D, H], fp32, kind="ExternalInput").ap()
    v_in = nc.dram_tensor("v", [H, L], fp32, kind="ExternalInput").ap()
    b_in = nc.dram_tensor("bias", [H], fp32, kind="ExternalInput").ap()
    out = nc.dram_tensor("out", [L, LD_LOC, LE], fp32, kind="ExternalOutput").ap()

    with tile.TileContext(nc) as tc:
        with (
            tc.tile_pool(name="const", bufs=1) as const_pool,
            tc.tile_pool(name="win", bufs=1) as win_pool,
            tc.tile_pool(name="proj", bufs=1) as proj_pool,
            tc.tile_pool(name="s", bufs=4) as s_pool,
            tc.tile_pool(name="tanh", bufs=4) as tanh_pool,
            tc.tile_pool(name="res", bufs=4) as res_pool,
        ):
            pro_ctx = tc.tile_pool(name="ps_pro", bufs=3, space="PSUM")
            pspro_pool = pro_ctx.__enter__()
            ident = const_pool.tile([128, 128], fp32)
            masks.make_identity(nc, ident[:])

            bias_sb = const_pool.tile([128, HC], fp32)
            nc.sync.dma_start(bias_sb[:], b_in.rearrange("(c p) -> p c", p=128))
            v_sb = const_pool.tile([128, HC * L], fp32)
            for c in range(HC):
                nc.sync.dma_start(
                    v_sb[:, c * L : (c + 1) * L], v_in[c * 128 : (c + 1) * 128, :]
                )

            v_mm = const_pool.tile([128, HC * 32], mmdt)
            nc.vector.memset(v_mm[:], 0.0)
            for c in range(HC):
                nc.vector.tensor_copy(
                    v_mm[:, c * 32 : c * 32 + L], v_sb[:, c * L : (c + 1) * L]
                )

            x_d_sb = win_pool.tile([128, D], fp32)
            nc.sync.dma_start(x_d_sb[:], x_d[:])
            x_e_sb = win_pool.tile([128, 2 * D], fp32)
            for eh in range(2):
                nc.sync.dma_start(
                    x_e_sb[:, eh * D : (eh + 1) * D], x_e[eh * 128 : (eh + 1) * 128, :]
                )
            w_d_sb = win_pool.tile([128, KC * H], fp32)
            w_e_sb = win_pool.tile([128, KC * H], fp32)
            for k in range(KC):
                nc.sync.dma_start(
                    w_d_sb[:, k * H : (k + 1) * H], w_d[k * 128 : (k + 1) * 128, :]
                )
                nc.sync.dma_start(
                    w_e_sb[:, k * H : (k + 1) * H], w_e[k * 128 : (k + 1) * 128, :]
                )

            # Transposes: x_dT[k][i, d], x_eT[k][i, e]
            x_dT = win_pool.tile([128, KC * 128], fp32)
            x_eT = win_pool.tile([128, KC * LE], fp32)
            for k in range(KC):
                pt = pspro_pool.tile([128, 256], fp32, tag="pro", name="pt")[:, :128]
                nc.tensor.transpose(
                    pt[:], x_d_sb[:, k * 128 : (k + 1) * 128], ident[:]
                )
                nc.vector.tensor_copy(x_dT[:, k * 128 : (k + 1) * 128], pt[:])
            for eh in range(2):
                for k in range(KC):
                    pt = pspro_pool.tile([128, 256], fp32, tag="pro", name="pt")[:, :128]
                    nc.tensor.transpose(
                        pt[:], x_e_sb[:, eh * D + k * 128 : eh * D + (k + 1) * 128],
                        ident[:],
                    )
                    nc.vector.tensor_copy(
                        x_eT[:, k * LE + eh * 128 : k * LE + (eh + 1) * 128], pt[:]
                    )

            # Projections: out_dT[c][h_l, d] and out_ebT[c][h_l, e] (+bias)
            out_dT = proj_pool.tile([128, HC * 128], fp32)
            out_ebT = proj_pool.tile([128, HC * LE], fp32)
            for c in range(HC):
                pd = pspro_pool.tile([128, 256], fp32, tag="pro", name="pd")[:, :128]
                for k in range(KC):
                    nc.tensor.matmul(
                        pd[:],
                        w_d_sb[:, k * H + c * 128 : k * H + (c + 1) * 128],
                        x_dT[:, k * 128 : (k + 1) * 128],
                        start=(k == 0),
                        stop=(k == KC - 1),
                    )
                nc.vector.tensor_copy(out_dT[:, c * 128 : (c + 1) * 128], pd[:])
            for c in range(HC):
                pe = pspro_pool.tile([128, 256], fp32, tag="pro", name="pe")
                for k in range(KC):
                    nc.tensor.matmul(
                        pe[:],
                        w_e_sb[:, k * H + c * 128 : k * H + (c + 1) * 128],
                        x_eT[:, k * LE : (k + 1) * LE],
                        start=(k == 0),
                        stop=(k == KC - 1),
                    )
                nc.scalar.activation(
                    out_ebT[:, c * LE : (c + 1) * LE],
                    pe[:],
                    mybir.ActivationFunctionType.Identity,
                    bias=bias_sb[:, c : c + 1],
                    scale=1.0,
                )

            pro_ctx.__exit__(None, None, None)
            pse_ctx = tc.tile_pool(name="ps_e", bufs=6, space="PSUM")
            pse_pool = pse_ctx.__enter__()

            # Steady loop over superblocks of 16 d
            n_big = SBLK - N_BIAS
            for sblk in range(N_SBLK):
                d0 = sblk * SBLK
                t_tiles = []
                for c in range(HC):
                    ebT = out_ebT[:, c * LE : (c + 1) * LE]
                    s_t = s_pool.tile([128, n_big * LE], fp32)
                    for i in range(n_big):
                        nc.vector.tensor_scalar_add(
                            s_t[:, i * LE : (i + 1) * LE],
                            ebT,
                            out_dT[:, c * 128 + d0 + i : c * 128 + d0 + i + 1],
                        )
                    t_t = tanh_pool.tile([128, SBLK * LE], mmdt)
                    nc.scalar.activation(
                        t_t[:, : n_big * LE],
                        s_t[:],
                        mybir.ActivationFunctionType.Tanh,
                    )
                    for i in range(n_big, SBLK):
                        nc.scalar.activation(
                            t_t[:, i * LE : (i + 1) * LE],
                            ebT,
                            mybir.ActivationFunctionType.Tanh,
                            bias=out_dT[:, c * 128 + d0 + i : c * 128 + d0 + i + 1],
                            scale=1.0,
                        )
                    t_tiles.append(t_t)

                for bank in range(2):
                    ps = pse_pool.tile([128, 512], fp32)
                    for j in range(4):
                        pair = bank * 4 + j
                        for c in range(HC):
                            nc.tensor.matmul(
                                ps[32 * j : 32 * j + 32, :],
                                v_mm[:, c * 32 : (c + 1) * 32],
                                t_tiles[c][:, pair * 512 : (pair + 1) * 512],
                                start=(c == 0),
                                stop=(c == HC - 1),
                                tile_position=(0, 32 * j),
                            )
                    res = res_pool.tile([128, 512], fp32)
                    nc.vector.tensor_copy(res[:], ps[:])
                    for j in range(4):
                        dd = d0 + bank * 8 + 2 * j
                        nc.sync.dma_start(
                            out[:, dd : dd + 2, :],
                            res[32 * j : 32 * j + L, :].rearrange(
                                "p (d e) -> p d e", e=LE
                            ),
                        )
            pse_ctx.__exit__(None, None, None)
    nc.compile()
    return nc


def _get_nc():
    if "nc" not in _CACHE:
        _CACHE["nc"] = _build_nc()
    return _CACHE["nc"]


def make_in_maps(input_d, input_e, W_d, W_e, b, v):
    input_d = np.asarray(input_d, np.float32)
    input_e = np.asarray(input_e, np.float32)
    W_d = np.asarray(W_d, np.float32)
    W_e = np.asarray(W_e, np.float32)
    b = np.asarray(b, np.float32)
    v = np.asarray(v, np.float32)
    in_maps = []
    for c in range(8):
        bi, dh = c // 2, c % 2
        in_maps.append(
            {
                "input_d": np.ascontiguousarray(
                    input_d[bi, dh * LD_LOC : (dh + 1) * LD_LOC, :]
                ),
                "input_e": np.ascontiguousarray(input_e[bi]),
                "w_d": W_d,
                "w_e": W_e,
                "v": v,
                "bias": b,
            }
        )
    return in_maps


def assemble(results):
    out = np.empty((B, L, LD, LE), np.float32)
    for c in range(8):
        bi, dh = c // 2, c % 2
        out[bi, :, dh * LD_LOC : (dh + 1) * LD_LOC, :] = results[c]["out"]
    return out


def kernel(input_d, input_e, W_d, W_e, b, v):
    from concourse.bass_utils import run_bass_kernel_spmd

    nc = _get_nc()
    in_maps = make_in_maps(input_d, input_e, W_d, W_e, b, v)
    res = run_bass_kernel_spmd(nc, in_maps, core_ids=list(range(8)))
    return assemble(res.results)
